# revision 1
# baseline (speedup 1.0000x reference)
"""Trainium2 Bass kernel for continuous-filter convolution (gnn message passing).

Reference computation (shapes hardcoded):
    features [2,256,32] f32, geometry [2,256,3] f32, centers [16] f32,
    kernel_w [16,32,32] f32, n_norm scalar
    d[z,a,b]   = sqrt(sum_c (g[z,b,c]-g[z,a,c])^2 + 1e-9)
    rbf        = exp(-10*(d[...,None]-centers)^2)            [z,a,b,n]
    k          = einsum('zabn,nij->zabij', rbf, kernel_w) / sqrt(n_norm)
    out[z,a,i] = einsum('zabij,zbj->zai', k, features)

Key restructuring: k is never materialized. Instead
    m[z,b,n,i]  = sum_j kernel_w[n,i,j] * features[z,b,j] / sqrt(n_norm)
    out[z,a,i]  = sum_{b,n} rbf[z,a,b,n] * m[z,b,n,i]
which is a [64 x 4096] @ [4096 x 32] contraction per (z, a-slice).

Sharding: 8 cores = 2 batches x 4 a-slices of 64 points. Each core gets its
geometry slice plus replicated features/weights; no cross-device reduction.

Per-core dataflow (b = point index, 2 chunks of 128 on partitions):
    d[b,(ch,a)]  direct-form distances on DVE (keeps the a==b diagonal exact)
    rbf[b,(n,a)] per (chunk, n-half) unit: DVE sub -> DVE/GpSimd square ->
                 ACT exp -> 8 accumulating PE matmuls (K=128)
    m[b,(n,i)]   two PE matmuls vs pre-transposed weights; PSUM->SBUF copies
                 split between ACT and DVE per n-half, placed off the
                 critical chain
"""

import numpy as np
from contextlib import ExitStack

import concourse.bass as bass
import concourse.tile as tile
from concourse import mybir
from concourse.bass_utils import run_bass_kernel_spmd

GAMMA = 10.0
EPS = 1e-9
B, P, C = 2, 256, 32
NB, I, J = 16, 32, 32
NCORES = 8
AS = NCORES // B  # a-slices per batch = 4
AL = P // AS      # points per a-slice = 64
NH = NB // 2      # n-half size = 8

f32 = mybir.dt.float32

# packed input A: [128, 215] = gab(192, broadcast) | gb6(6) | cb(16) | eps(1)
GA0, GB0, CB0, EPS0 = 0, 192, 198, 214
WA = 215
# packed input B: [32, 768] = ft(256) | wt(512)
FT0, WT0 = 0, 256
WB = 768


def _split_multi_waits(nc):
    """This walrus build only lowers one sync wait per instruction; Tile's
    scheduler attaches several to some instructions (notably the tail drain).
    Hoist extras into single-wait EventSemaphore instructions just before, on
    the same engine — semantically identical, sequencer waits then issues."""
    n = 0
    for fn in nc.m.functions:
        for bb in fn.blocks:
            insts = list(bb.instructions)
            new = []
            for inst in insts:
                si = getattr(inst, "sync_info", None)
                if si is not None and si.on_wait and len(si.on_wait) > 1:
                    waits = list(si.on_wait)
                    for w in waits[:-1]:
                        n += 1
                        new.append(
                            mybir.InstEventSemaphore(
                                name=f"I-msplit{n}",
                                engine=inst.engine,
                                sync_info=mybir.SyncInfo(on_wait=[w], on_update=[]),
                            )
                        )
                    inst.sync_info = mybir.SyncInfo(
                        on_wait=[waits[-1]], on_update=list(si.on_update or [])
                    )
                new.append(inst)
            try:
                bb.instructions = new
            except Exception:
                bb.instructions.clear()
                for i in new:
                    bb.add_instruction(i)
    return n


def _build_program():
    nc = bass.Bass(debug=False)
    g_a = nc.declare_dram_parameter("ina", [128, WA], f32, isOutput=False)
    g_b = nc.declare_dram_parameter("inb", [J, WB], f32, isOutput=False)
    g_out = nc.declare_dram_parameter("out", [AL, I], f32, isOutput=True)

    Act = mybir.ActivationFunctionType
    Alu = mybir.AluOpType
    const0 = nc.const_aps.aps[(f32, 0.0)]

    with ExitStack() as ctx:
        tc = ctx.enter_context(tile.TileContext(nc))
        pool = ctx.enter_context(tc.tile_pool(name="sb", bufs=1))
        pipe = ctx.enter_context(tc.tile_pool(name="pipe", bufs=2))
        ppool = ctx.enter_context(tc.tile_pool(name="ps", bufs=1, space="PSUM"))

        # warm the ACT sqrt table while DMAs are in flight (exp's table loads
        # in the idle window between the sqrt and the first exp)
        junk = pool.tile([128, 1], f32, tag="junk")
        nc.scalar.activation(junk[:], const0[:, 0:1], Act.Sqrt)

        t_a = pool.tile([128, WA], f32, tag="ina")
        nc.sync.dma_start(t_a[:], g_a[:])
        t_b = pool.tile([J, WB], f32, tag="inb")
        nc.scalar.dma_start(t_b[:], g_b[:])

        ga3 = t_a[:, GA0 : GA0 + AL * 3].rearrange("p (a c) -> p a c", c=3)
        gb3 = t_a[:, GB0 : GB0 + 6].rearrange("p (h c) -> p h c", c=3)
        cb = t_a[:, CB0 : CB0 + NB]
        epsc = t_a[:, EPS0 : EPS0 + 1]

        # distance chain, split per chunk so both Sqrt activations retire
        # before the Exp table load starts: d[b, (ch, a)]
        d = pool.tile([128, 2 * AL], f32, tag="d")
        for ch in range(2):
            diff = pipe.tile([128, AL * 3], f32, tag="diff")
            diff3 = diff[:].rearrange("p (a c) -> p a c", c=3)
            nc.vector.tensor_sub(
                diff3,
                ga3,
                gb3[:, ch, :].unsqueeze(1).broadcast_to([128, AL, 3]),
            )
            sqd = pipe.tile([128, AL * 3], f32, tag="sqd")
            sqd3 = sqd[:].rearrange("p (a c) -> p a c", c=3)
            nc.vector.tensor_mul(sqd3, diff3, diff3)
            d2 = pipe.tile([128, AL], f32, tag="d2")
            nc.vector.tensor_reduce(
                d2[:], sqd3, axis=mybir.AxisListType.X, op=Alu.add
            )
            nc.scalar.activation(
                d[:, ch * AL : (ch + 1) * AL], d2[:], Act.Sqrt, bias=epsc
            )

        # m[b, (n,i)] per chunk, quartered so the first PSUM half is ready
        # early for its SBUF copy
        HW2 = NH * I  # 256
        pm = []
        for ch in range(2):
            p = ppool.tile([128, NB * I], f32, tag=f"pm{ch}", name=f"pm{ch}")
            for h in range(2):
                nc.tensor.matmul(
                    p[:, h * HW2 : (h + 1) * HW2],
                    lhsT=t_b[:, FT0 + ch * 128 : FT0 + (ch + 1) * 128],
                    rhs=t_b[:, WT0 + h * HW2 : WT0 + (h + 1) * HW2],
                    start=True,
                    stop=True,
                )
            pm.append(p)
        t_m = [
            pool.tile([128, NB * I], f32, tag=f"m{ch}", name=f"m{ch}")
            for ch in range(2)
        ]
        HW = NH * I  # columns per n-half = 256

        # rbf + contraction, pipelined in 4 units of (chunk, n-half).
        # PSUM->SBUF m copies are interleaved: each unit's m half is copied
        # just before it is needed, alternating DVE/ACT.
        po = ppool.tile([AL, I], f32, tag="po")
        first = True
        for ch in range(2):
            for h in range(2):
                u = ch * 2 + h
                tt = pipe.tile([128, NH * AL], f32, tag="tt")
                nc.vector.tensor_sub(
                    tt[:].rearrange("p (n a) -> p n a", n=NH),
                    d[:, ch * AL : (ch + 1) * AL]
                    .unsqueeze(1)
                    .broadcast_to([128, NH, AL]),
                    cb[:, h * NH : (h + 1) * NH]
                    .unsqueeze(2)
                    .broadcast_to([128, NH, AL]),
                )
                sq2 = pipe.tile([128, NH * AL], f32, tag="sq2")
                if u == 1:
                    nc.gpsimd.tensor_mul(sq2[:], tt[:], tt[:])
                else:
                    nc.vector.tensor_mul(sq2[:], tt[:], tt[:])
                # copy this unit's m half on ACT just before its exp — ACT is
                # idle there, and DVE placement delayed the first contraction
                nc.scalar.copy(
                    t_m[ch][:, h * HW : (h + 1) * HW],
                    pm[ch][:, h * HW : (h + 1) * HW],
                )
                rbf = pipe.tile([128, NH * AL], f32, tag="rbf")
                nc.scalar.activation(rbf[:], sq2[:], Act.Exp, scale=-GAMMA)
                for k in range(NH):
                    n = h * NH + k
                    nc.tensor.matmul(
                        po[:],
                        lhsT=rbf[:, k * AL : (k + 1) * AL],
                        rhs=t_m[ch][:, n * I : (n + 1) * I],
                        start=first,
                        stop=(ch == 1 and n == NB - 1),
                    )
                    first = False
        t_o = pool.tile([AL, I], f32, tag="o")
        nc.scalar.copy(t_o[:], po[:])
        nc.sync.dma_start(g_out[:], t_o[:])

    _split_multi_waits(nc)
    return nc


_NC = None


def _pack_inputs(features, geometry, centers, kernel_w, n_norm):
    features = np.asarray(features, np.float32)
    geometry = np.asarray(geometry, np.float32)
    centers = np.asarray(centers, np.float32)
    kernel_w = np.asarray(kernel_w, np.float32)
    scale = 1.0 / np.sqrt(float(np.asarray(n_norm).item()))

    wt = np.ascontiguousarray(kernel_w.transpose(2, 0, 1).reshape(J, NB * I))
    in_maps = []
    for core in range(NCORES):
        z, sl = divmod(core, AS)
        ina = np.empty((128, WA), np.float32)
        ina[:, GA0 : GA0 + AL * 3] = geometry[z, sl * AL : (sl + 1) * AL, :].reshape(
            1, AL * 3
        )
        ina[:, GB0 : GB0 + 6] = (
            geometry[z].reshape(2, 128, 3).transpose(1, 0, 2).reshape(128, 6)
        )
        ina[:, CB0 : CB0 + NB] = centers.reshape(1, NB)
        ina[:, EPS0] = EPS
        inb = np.empty((J, WB), np.float32)
        inb[:, FT0 : FT0 + P] = features[z].T * scale
        inb[:, WT0 : WT0 + NB * I] = wt
        in_maps.append({"ina": ina, "inb": inb})
    return in_maps


def kernel(features, geometry, centers, kernel_w, n_norm):
    global _NC
    if _NC is None:
        _NC = _build_program()

    in_maps = _pack_inputs(features, geometry, centers, kernel_w, n_norm)
    res = run_bass_kernel_spmd(_NC, in_maps, list(range(NCORES)))

    out = np.empty((B, P, I), np.float32)
    for core in range(NCORES):
        z, sl = divmod(core, AS)
        out[z, sl * AL : (sl + 1) * AL, :] = res.results[core]["out"]
    return out



# revision 5
# speedup vs baseline: 1.0650x; 1.0650x over previous
"""Trainium2 Bass kernel for continuous-filter convolution (gnn message passing).

Reference computation (shapes hardcoded):
    features [2,256,32] f32, geometry [2,256,3] f32, centers [16] f32,
    kernel_w [16,32,32] f32, n_norm scalar
    d[z,a,b]   = sqrt(sum_c (g[z,b,c]-g[z,a,c])^2 + 1e-9)
    rbf        = exp(-10*(d[...,None]-centers)^2)            [z,a,b,n]
    k          = einsum('zabn,nij->zabij', rbf, kernel_w) / sqrt(n_norm)
    out[z,a,i] = einsum('zabij,zbj->zai', k, features)

Restructuring (v2):
    d^2 via one augmented K=5 PE matmul per b-chunk:
        d2[b,a] = sum_t gb'[t,b]*ga'[t,a],  gb'=[-2g,1,|g|^2], ga'=[g,|g|^2,1]
    rbf[b,(n,a)] = exp(-g*(d[b,a]-c_n)^2) in bf16 (DVE sub/sq + ACT exp;
        centers pre-replicated over `a` host-side so DVE ops hit 2x mode)
    stage 1:  T'[j,(n,a)]  = sum_b f[b,j]*rbf[b,(n,a)]      (bf16 PE, N=512)
    transpose T' -> T''[(g,j),(k,a)] with 4 contiguous PSUM->SBUF copies
        (n = 4g+k), casting to bf16
    stage 2:  out[a,i]     = sum_k sum_{(g,j)} T''[.,(k,a)]*W2[.,(k,i)]
        with W2[(g,j),(k,i)] = w[4g+k,i,j]/sqrt(n_norm)      (bf16 PE, K=128)

Sharding: 8 cores = 2 batches x 4 a-slices of 64 points; features/weights
replicated, no cross-device reduction.

ACT's two activation tables (sqrt+exp) are warmed by junk activations at
program start so neither 1.3us table load sits on the critical path.
"""

import numpy as np
from contextlib import ExitStack

import ml_dtypes
import concourse.bass as bass
import concourse.tile as tile
from concourse import mybir
from concourse.bass_utils import run_bass_kernel_spmd

GAMMA = 10.0
EPS_D2 = 5e-5  # sqrt bias; absorbs PE rounding residual on the a==b diagonal
B, P, C = 2, 256, 32
NB, I, J = 16, 32, 32
NCORES = 8
AS = NCORES // B  # a-slices per batch = 4
AL = P // AS      # points per a-slice = 64

f32 = mybir.dt.float32
bf16 = mybir.dt.bfloat16
npbf16 = ml_dtypes.bfloat16


def _split_multi_waits(nc):
    """This walrus build only lowers one sync wait per instruction; Tile's
    scheduler attaches several to some instructions (notably the tail drain).
    Hoist extras into single-wait EventSemaphore instructions just before, on
    the same engine — semantically identical, sequencer waits then issues."""
    n = 0
    for fn in nc.m.functions:
        for bb in fn.blocks:
            insts = list(bb.instructions)
            new = []
            for inst in insts:
                si = getattr(inst, "sync_info", None)
                if si is not None and si.on_wait and len(si.on_wait) > 1:
                    waits = list(si.on_wait)
                    for w in waits[:-1]:
                        n += 1
                        new.append(
                            mybir.InstEventSemaphore(
                                name=f"I-msplit{n}",
                                engine=inst.engine,
                                sync_info=mybir.SyncInfo(on_wait=[w], on_update=[]),
                            )
                        )
                    inst.sync_info = mybir.SyncInfo(
                        on_wait=[waits[-1]], on_update=list(si.on_update or [])
                    )
                new.append(inst)
            try:
                bb.instructions = new
            except Exception:
                bb.instructions.clear()
                for i in new:
                    bb.add_instruction(i)
    return n


def _build_program():
    nc = bass.Bass(debug=False)
    g_g = nc.declare_dram_parameter("ing", [5, 320], f32, isOutput=False)
    g_cbx = nc.declare_dram_parameter("incb", [128, NB * AL], bf16, isOutput=False)
    g_fw = nc.declare_dram_parameter("infw", [128, 192], bf16, isOutput=False)
    g_out = nc.declare_dram_parameter("out", [AL, I], f32, isOutput=True)

    Act = mybir.ActivationFunctionType
    const0 = nc.const_aps.aps[(f32, 0.0)]

    with ExitStack() as ctx:
        tc = ctx.enter_context(tile.TileContext(nc))
        pool = ctx.enter_context(tc.tile_pool(name="sb", bufs=1))
        ppool = ctx.enter_context(tc.tile_pool(name="ps", bufs=1, space="PSUM"))

        # warm both ACT tables (sqrt, exp) while DMAs are in flight
        junk = pool.tile([128, 1], f32, tag="junk")
        nc.scalar.activation(junk[:], const0[:, 0:1], Act.Sqrt)
        junk2 = pool.tile([128, 1], f32, tag="junk2")
        nc.scalar.activation(junk2[:], const0[:, 0:1], Act.Exp)

        # inputs; issue on different engines so the DMAs overlap
        t_g = pool.tile([5, 320], f32, tag="g")
        nc.gpsimd.dma_start(t_g[:], g_g[:])
        t_cbx = pool.tile([128, NB * AL], bf16, tag="cbx")
        nc.gpsimd.dma_start(t_cbx[:, 0:512], g_cbx[:, 0:512])
        nc.sync.dma_start(t_cbx[:, 512:1024], g_cbx[:, 512:1024])
        t_fw = pool.tile([128, 192], bf16, tag="fw")
        nc.sync.dma_start(t_fw[:], g_fw[:])

        t_eps = pool.tile([128, 1], f32, tag="eps")
        nc.gpsimd.memset(t_eps[:], EPS_D2)

        # PSUM: T' first so its two matmul dsts are bank-aligned
        p_T = ppool.tile([J, NB * AL], f32, tag="pT", name="pT")
        p_d = ppool.tile([128, 512], f32, tag="pd", name="pd")  # bank-padded
        p_o = ppool.tile([AL, I], f32, tag="po", name="po")

        # d^2 via augmented matmul, then one sqrt -> d bf16 [128,(ch,a)]
        for ch in range(2):
            nc.tensor.matmul(
                p_d[:, ch * AL : (ch + 1) * AL],
                lhsT=t_g[:, ch * 128 : (ch + 1) * 128],
                rhs=t_g[:, 256:320],
                start=True,
                stop=True,
            )
        t_d = pool.tile([128, 2 * AL], bf16, tag="d")
        nc.scalar.activation(t_d[:], p_d[:, 0 : 2 * AL], Act.Sqrt, bias=t_eps[:])

        t_t = pool.tile([128, 2048], bf16, tag="tt")
        t_q = pool.tile([128, 2048], bf16, tag="sq")
        t_r = pool.tile([128, 2048], bf16, tag="rbf")
        cb3 = t_cbx[:].rearrange("p (n a) -> p n a", a=AL)
        for ch in range(2):
            sl = slice(ch * 1024, (ch + 1) * 1024)
            tt3 = t_t[:, sl].rearrange("p (n a) -> p n a", a=AL)
            nc.vector.tensor_sub(
                tt3,
                t_d[:, ch * AL : (ch + 1) * AL]
                .unsqueeze(1)
                .broadcast_to([128, NB, AL]),
                cb3,
            )
            nc.vector.tensor_mul(t_q[:, sl], t_t[:, sl], t_t[:, sl])
            nc.scalar.activation(t_r[:, sl], t_q[:, sl], Act.Exp, scale=-GAMMA)
            # stage 1: T'[j,(n,a)] += f_ch^T @ rbf_ch, one matmul per n-half
            for h in range(2):
                nc.tensor.matmul(
                    p_T[:, h * 512 : (h + 1) * 512],
                    lhsT=t_fw[:, ch * J : (ch + 1) * J],
                    rhs=t_r[:, ch * 1024 + h * 512 : ch * 1024 + (h + 1) * 512],
                    start=(ch == 0),
                    stop=(ch == 1),
                )

        # transpose T'[j, n*64+a] -> T''[(g,j), (k,a)] with n = 4g+k (bf16).
        # per g the source block is contiguous, so each copy is one AP.
        t_tp = pool.tile([128, 256], bf16, tag="tp")
        copy_eng = [nc.vector, nc.scalar, nc.scalar, nc.vector]
        for g in range(4):
            eng = copy_eng[g]
            src = p_T[:, 256 * g : 256 * (g + 1)]
            dst = t_tp[32 * g : 32 * (g + 1), :]
            if eng is nc.scalar:
                nc.scalar.copy(dst, src)
            else:
                nc.vector.tensor_scalar_add(dst, src, 0.0)

        # stage 2: out[a,i] = sum_k T''[:, (k,a)]^T @ W2[:, (k,i)]
        for k in range(4):
            nc.tensor.matmul(
                p_o[:],
                lhsT=t_tp[:, k * AL : (k + 1) * AL],
                rhs=t_fw[:, 64 + k * I : 64 + (k + 1) * I],
                start=(k == 0),
                stop=(k == 3),
            )
        t_o = pool.tile([AL, I], f32, tag="o")
        nc.vector.tensor_scalar_add(t_o[:], p_o[:], 0.0)
        nc.sync.dma_start(g_out[:], t_o[:])

    _split_multi_waits(nc)
    return nc


_NC = None


def _pack_inputs(features, geometry, centers, kernel_w, n_norm):
    features = np.asarray(features, np.float32)
    geometry = np.asarray(geometry, np.float32)
    centers = np.asarray(centers, np.float32)
    kernel_w = np.asarray(kernel_w, np.float32)
    scale = 1.0 / np.sqrt(float(np.asarray(n_norm).item()))

    cbx = np.broadcast_to(
        np.repeat(centers, AL)[None, :], (128, NB * AL)
    ).astype(npbf16)
    in_maps = []
    for core in range(NCORES):
        z, sl = divmod(core, AS)
        geo = geometry[z]  # [256, 3]
        gsq = (geo * geo).sum(1)
        gpack = np.empty((5, 320), np.float32)
        gpack[0:3, 0:256] = -2.0 * geo.T
        gpack[3, 0:256] = 1.0
        gpack[4, 0:256] = gsq
        ga = geo[sl * AL : (sl + 1) * AL]
        gpack[0:3, 256:320] = ga.T
        gpack[3, 256:320] = (ga * ga).sum(1)
        gpack[4, 256:320] = 1.0

        fw = np.empty((128, 192), np.float32)
        fw[:, 0:64] = (
            features[z].reshape(2, 128, C).transpose(1, 0, 2).reshape(128, 64)
        )
        # W2[(g,j),(k,i)] = w[4g+k, i, j] * scale
        fw[:, 64:192] = (
            kernel_w.reshape(4, 4, I, J).transpose(0, 3, 1, 2).reshape(128, 128)
            * scale
        )
        in_maps.append(
            {"ing": gpack, "incb": cbx, "infw": fw.astype(npbf16)}
        )
    return in_maps


def kernel(features, geometry, centers, kernel_w, n_norm):
    global _NC
    if _NC is None:
        _NC = _build_program()

    in_maps = _pack_inputs(features, geometry, centers, kernel_w, n_norm)
    res = run_bass_kernel_spmd(_NC, in_maps, list(range(NCORES)))

    out = np.empty((B, P, I), np.float32)
    for core in range(NCORES):
        z, sl = divmod(core, AS)
        out[z, sl * AL : (sl + 1) * AL, :] = res.results[core]["out"]
    return out


# revision 10
# speedup vs baseline: 1.1604x; 1.0896x over previous
"""Trainium2 Bass kernel for continuous-filter convolution (gnn message passing).

Reference computation (shapes hardcoded):
    features [2,256,32] f32, geometry [2,256,3] f32, centers [16] f32,
    kernel_w [16,32,32] f32, n_norm scalar
    d[z,a,b]   = sqrt(sum_c (g[z,b,c]-g[z,a,c])^2 + 1e-9)
    rbf        = exp(-10*(d[...,None]-centers)^2)            [z,a,b,n]
    k          = einsum('zabn,nij->zabij', rbf, kernel_w) / sqrt(n_norm)
    out[z,a,i] = einsum('zabij,zbj->zai', k, features)

Restructuring (v3, fp16 datapath):
    d^2 via one augmented K=5 fp32 PE matmul per 128-point b-chunk:
        d2[b,a] = sum_t gb'[t,b]*ga'[t,a], gb'=[-2g,1,|g|^2], ga'=[g,|g|^2,1]
        (bias 5e-5 under the sqrt absorbs the PE rounding residual on the
        a==b diagonal; fp32 residual is ~2e-6)
    d = sqrt(d2+eps) -> fp16; rbf[b,(n,a)] = exp(-g*(d-c_n)^2) in fp16
        (centers pre-replicated over `a` host-side -> DVE 2x mode; fp16 not
        bf16 because quantization of d/c scales with d and hits exactly the
        pairs where rbf is large — fp16 keeps absmax rel err ~2e-3)
    m[b,(n,i)] = sum_j f[b,j] w[n,i,j]/sqrt(n_norm): one fp16 K=32 PE matmul
        per chunk (N=512), PSUM->SBUF fp16 copies on gpsimd/DVE off the
        critical path
    out[a,i] = sum_{b,n} rbf[b,(n,a)] m[b,(n,i)]: 32 accumulating fp16
        matmuls (K=128, stationary=rbf slice, moving=m slice), ch0's 16
        overlap exp of ch1.

Sharding: 8 cores = 2 batches x 4 a-slices of 64 points; features/weights
replicated, no cross-device reduction.

ACT ordering: sqrt table loads eagerly at engine start (dep-free), exp
table load follows the sqrt; ACT does only sqrt+2 exps. Any extra
activation-function switch costs a 1283ns table reload, so no junk warmups.
"""

import numpy as np
from contextlib import ExitStack

import concourse.bass as bass
import concourse.tile as tile
from concourse import mybir
from concourse.bass_utils import run_bass_kernel_spmd

GAMMA = 10.0
EPS_D2 = 5e-5
B, P, C = 2, 256, 32
NB, I, J = 16, 32, 32
NCORES = 8
AS = NCORES // B  # a-slices per batch = 4
AL = P // AS      # points per a-slice = 64

f32 = mybir.dt.float32
f16 = mybir.dt.float16


def _split_multi_waits(nc):
    """This walrus build only lowers one sync wait per instruction; Tile's
    scheduler attaches several to some instructions (notably the tail drain).
    Hoist extras into single-wait EventSemaphore instructions just before, on
    the same engine — semantically identical, sequencer waits then issues."""
    n = 0
    for fn in nc.m.functions:
        for bb in fn.blocks:
            insts = list(bb.instructions)
            new = []
            for inst in insts:
                si = getattr(inst, "sync_info", None)
                if si is not None and si.on_wait and len(si.on_wait) > 1:
                    waits = list(si.on_wait)
                    for w in waits[:-1]:
                        n += 1
                        new.append(
                            mybir.InstEventSemaphore(
                                name=f"I-msplit{n}",
                                engine=inst.engine,
                                sync_info=mybir.SyncInfo(on_wait=[w], on_update=[]),
                            )
                        )
                    inst.sync_info = mybir.SyncInfo(
                        on_wait=[waits[-1]], on_update=list(si.on_update or [])
                    )
                new.append(inst)
            try:
                bb.instructions = new
            except Exception:
                bb.instructions.clear()
                for i in new:
                    bb.add_instruction(i)
    return n


def _build_program():
    nc = bass.Bass(debug=False)
    g_g = nc.declare_dram_parameter("ing", [5, 320], f32, isOutput=False)
    g_cbx = nc.declare_dram_parameter("incb", [128, NB * AL], f16, isOutput=False)
    g_fw = nc.declare_dram_parameter("infw", [J, 768], f16, isOutput=False)
    g_out = nc.declare_dram_parameter("out", [AL, I], f32, isOutput=True)

    Act = mybir.ActivationFunctionType

    with ExitStack() as ctx:
        tc = ctx.enter_context(tile.TileContext(nc))
        pool = ctx.enter_context(tc.tile_pool(name="sb", bufs=1))
        ppool = ctx.enter_context(tc.tile_pool(name="ps", bufs=1, space="PSUM"))

        # inputs; spread across SP + gpsimd so the DMAs overlap
        t_g = pool.tile([5, 320], f32, tag="g")
        nc.sync.dma_start(t_g[:], g_g[:])
        t_cbx = pool.tile([128, NB * AL], f16, tag="cbx")
        nc.sync.dma_start(t_cbx[:, 512:1024], g_cbx[:, 512:1024])
        t_fw = pool.tile([J, 768], f16, tag="fw")
        nc.sync.dma_start(t_fw[:], g_fw[:])
        nc.gpsimd.dma_start(t_cbx[:, 0:512], g_cbx[:, 0:512])

        t_eps = pool.tile([128, 1], f32, tag="eps")
        nc.vector.memset(t_eps[:], EPS_D2)

        # PSUM banks: m0, m1, d2 (padded), out
        p_m = [
            ppool.tile([128, 512], f32, tag=f"pm{ch}", name=f"pm{ch}")
            for ch in range(2)
        ]
        p_d = ppool.tile([128, 512], f32, tag="pd", name="pd")
        p_o = ppool.tile([AL, I], f32, tag="po", name="po")

        # d^2 via augmented matmul, then one sqrt -> d fp16 [128,(ch,a)]
        for ch in range(2):
            nc.tensor.matmul(
                p_d[:, ch * AL : (ch + 1) * AL],
                lhsT=t_g[:, ch * 128 : (ch + 1) * 128],
                rhs=t_g[:, 256:320],
                start=True,
                stop=True,
            )
        t_d = pool.tile([128, 2 * AL], f16, tag="d")
        nc.scalar.activation(t_d[:], p_d[:, 0 : 2 * AL], Act.Sqrt, bias=t_eps[:])

        # m[b,(n,i)] per chunk: K=32 fp16 matmul, then PSUM->SBUF fp16 copy
        for ch in range(2):
            nc.tensor.matmul(
                p_m[ch][:],
                lhsT=t_fw[:, ch * 128 : (ch + 1) * 128],
                rhs=t_fw[:, 256:768],
                start=True,
                stop=True,
            )
        t_m = [
            pool.tile([128, 512], f16, tag=f"m{ch}", name=f"m{ch}")
            for ch in range(2)
        ]

        # rbf + contraction, per chunk; ch0's matmuls overlap ch1's exp
        t_t = pool.tile([128, 2048], f16, tag="tt")
        t_q = pool.tile([128, 2048], f16, tag="sq")
        t_r = pool.tile([128, 2048], f16, tag="rbf")
        cb3 = t_cbx[:].rearrange("p (n a) -> p n a", a=AL)
        first = True
        for ch in range(2):
            sl = slice(ch * 1024, (ch + 1) * 1024)
            tt3 = t_t[:, sl].rearrange("p (n a) -> p n a", a=AL)
            nc.vector.tensor_sub(
                tt3,
                t_d[:, ch * AL : (ch + 1) * AL]
                .unsqueeze(1)
                .broadcast_to([128, NB, AL]),
                cb3,
            )
            nc.vector.tensor_mul(t_q[:, sl], t_t[:, sl], t_t[:, sl])
            nc.scalar.activation(t_r[:, sl], t_q[:, sl], Act.Exp, scale=-GAMMA)
            nc.vector.tensor_scalar_add(t_m[ch][:], p_m[ch][:], 0.0)
            for n in range(NB):
                nc.tensor.matmul(
                    p_o[:],
                    lhsT=t_r[:, ch * 1024 + n * AL : ch * 1024 + (n + 1) * AL],
                    rhs=t_m[ch][:, n * I : (n + 1) * I],
                    start=first,
                    stop=(ch == 1 and n == NB - 1),
                )
                first = False

        t_o = pool.tile([AL, I], f32, tag="o")
        nc.vector.tensor_scalar_add(t_o[:], p_o[:], 0.0)
        nc.sync.dma_start(g_out[:], t_o[:])

    _split_multi_waits(nc)
    return nc


_NC = None


def _pack_inputs(features, geometry, centers, kernel_w, n_norm):
    features = np.asarray(features, np.float32)
    geometry = np.asarray(geometry, np.float32)
    centers = np.asarray(centers, np.float32)
    kernel_w = np.asarray(kernel_w, np.float32)
    scale = 1.0 / np.sqrt(float(np.asarray(n_norm).item()))

    cbx = np.broadcast_to(
        np.repeat(centers, AL)[None, :], (128, NB * AL)
    ).astype(np.float16)
    # w2[j, 32n+i] = w[n,i,j]*scale
    w2 = (kernel_w.transpose(2, 0, 1).reshape(J, NB * I) * scale).astype(
        np.float16
    )
    in_maps = []
    for core in range(NCORES):
        z, sl = divmod(core, AS)
        geo = geometry[z]  # [256, 3]
        gsq = (geo * geo).sum(1)
        gpack = np.empty((5, 320), np.float32)
        gpack[0:3, 0:256] = -2.0 * geo.T
        gpack[3, 0:256] = 1.0
        gpack[4, 0:256] = gsq
        ga = geo[sl * AL : (sl + 1) * AL]
        gpack[0:3, 256:320] = ga.T
        gpack[3, 256:320] = (ga * ga).sum(1)
        gpack[4, 256:320] = 1.0

        fw = np.empty((J, 768), np.float16)
        fw[:, 0:256] = features[z].T.astype(np.float16)
        fw[:, 256:768] = w2
        in_maps.append({"ing": gpack, "incb": cbx, "infw": fw})
    return in_maps


def kernel(features, geometry, centers, kernel_w, n_norm):
    global _NC
    if _NC is None:
        _NC = _build_program()

    in_maps = _pack_inputs(features, geometry, centers, kernel_w, n_norm)
    res = run_bass_kernel_spmd(_NC, in_maps, list(range(NCORES)))

    out = np.empty((B, P, I), np.float32)
    for core in range(NCORES):
        z, sl = divmod(core, AS)
        out[z, sl * AL : (sl + 1) * AL, :] = res.results[core]["out"]
    return out
